# revision 34
# baseline (speedup 1.0000x reference)
"""Trainium2 Bass kernel for nn_Decoder_64012192580153 (GNN pairwise decoder).

    pred[i, j] = sigmoid(W2 . relu(W1 @ [Z[i]; Z[j]] + b1) + b2),  Z: [2048, 32]

Math refactor (identical to the reference): A = Z @ W1[:D] + b1, B = Z @ W1[D:]
(tiny [N, H] mats, computed on host), then per output element
    pred[i, j] = sigmoid(sum_h W2[h] * relu(A[i, h] + B[j, h]) + b2).

Device strategy (8-way row-parallel; core c owns output rows [256c, 256c+256)):
  * Brep [128, N] fp16: B^T stacked twice on partitions (k = 2 rows x 64 hidden).
  * Per row-pair one fused op builds R[k, j] = relu(Brep[k, j] + a2t[k, pair])
    ([128, 2048] fp16): DVE tensor_scalar(add, max) in 4x perf mode, with a
    fraction of pairs on ACT (activation Relu w/ per-partition bias) to use
    both engines.
  * Reduction over k on the PE: zero-padded fp16 weight slots map each pair's
    two rows into distinct PSUM partitions; 4 matmuls per pair (512-col
    j-tiles) with tile_position col-groups so 4 pairs run concurrently in the
    128x128 array. 64 pairs accumulate into four per-bank [128, 512] f32 PSUM
    tiles per block (separate tiles so each bank's sigmoid depends only on its
    own stop-matmuls, shortening the drain).
  * Per-bank ACT Sigmoid (bias=b2) PSUM -> SBUF, then a 256 KB DMA store each.
  * Startup: input DMAs are split across the two HWDGE queues (scalar+sync)
    and hoisted above the engine-boot barrier so transfers overlap boot.

Measured on HW: ~82us (baseline 87.6us).  Breakdown: ~10us boot+input load
(fixed ~6.5us NRT/engine boot + ~2us HBM transfer + slop), ~64.4us body --
exactly the DVE+ACT elementwise floor (DVE tensor_scalar [128, 2048] fp16 4x
= 663ns/pair, ACT activate = 1990ns/pair, 97/31 split + 8 sigmoids), ~7.3us
tail (last matmuls + 4-sigmoid ACT chain + stores + final barrier).  The body
cannot go lower without a different algorithm: every relu'd element must pass
through DVE (4 elem/lane/cyc @0.96GHz ceiling) or ACT (1 elem/lane/cyc
@1.2GHz); GPSIMD shares DVE's SBUF ports; PE-only reformulations (polynomial /
separable approx of the relu kink) need contraction K ~ 64*(1/eps) >> 1664 for
the 2e-2 tolerance and lose.
"""

import sys

if "/opt/trn_rl_repo" not in sys.path:
    sys.path.insert(0, "/opt/trn_rl_repo")

import numpy as np

import concourse.bass as bass
import concourse.tile as tile
import concourse.mybir as mybir
from concourse.bass_utils import run_bass_kernel_spmd

N = 2048
D = 32
H = 64
NCORES = 8
RPC = N // NCORES          # rows per core (256)
NBLK = RPC // 128          # row blocks of 128 per core (2)
NPAIR = 64                 # row-pairs per block
JT = 512                   # j-tile width (one PSUM bank of f32)
NJT = N // JT              # j-tiles (4)
NQ = NPAIR // 4            # quad rounds per block (16)
# Quads (of 16 per block) whose ACT slot stays on DVE.  Block 0 gives ACT all
# 16 quads (no sigmoids compete there -- block-0 sigmoids run during block 1);
# block 1 drops one quad to make room for block-0's 4 sigmoids.
_ACT_SKIP_QUADS = [set(), {11}]

FP16 = mybir.dt.float16
F32 = mybir.dt.float32

# pair p of a block -> its first local output row (PSUM partition).
# p = 4q + g: col-group g = p % 4, accumulation slot s = p // 4.
_PAIR_ROW0 = [32 * (p % 4) + 2 * (p // 4) for p in range(NPAIR)]


def _use_act(b: int, p: int) -> bool:
    # One ACT pair per quad (always col-group 0) keeps the pipeline regular.
    q, g = p // 4, p % 4
    return g == 0 and q not in _ACT_SKIP_QUADS[b]


# This walrus build caps the sync-wait commands one instruction may carry
# (1 for CTRL-class e.g. Drain; small for compute classes).  Excess waits are
# moved onto same-engine NoOp instructions placed immediately before the
# over-limit instruction; engine program order preserves the semantics.
_WAIT_CAPS = {"InstDrain": 1, "default": 1}


def _split_sync_waits(nc):
    for fn in nc.m.functions:
        for bb in fn.blocks:
            out = []
            for ins in bb.instructions:
                si = ins.sync_info
                cap = _WAIT_CAPS.get(type(ins).__name__, _WAIT_CAPS["default"])
                if si is not None and si.on_wait and len(si.on_wait) > cap:
                    waits = list(si.on_wait)
                    head, tail = waits[:-cap], waits[-cap:]
                    for k, w in enumerate(head):
                        helper = mybir.InstNoOp(
                            name=f"{ins.name}-ws{k}", ins=[], outs=[]
                        )
                        helper.engine = ins.engine
                        helper.sync_info = mybir.SyncInfo(
                            on_wait=[w], on_update=[]
                        )
                        out.append(helper)
                    si.on_wait = tail
                out.append(ins)
            bb.instructions[:] = out


def _hoist_input_dmas(nc):
    """Move the leading wait-free input-DMA descriptors (any engine) to the
    very top of the main block, ahead of the engine preamble TENSOR_LOADs and
    the TileContext start barrier.  The issuing engines start executing within
    ~0.3us of kernel start, so the input transfers run during the ~6.5us
    engine-boot/barrier window instead of after it."""
    fn = nc.m.functions[0]
    main_bb, tile_bb = fn.blocks[0], fn.blocks[1]
    hoist, rest = [], []
    for ins in tile_bb.instructions:
        if (
            len(rest) < 8
            and type(ins).__name__ == "InstDMACopy"
            and not (ins.sync_info and ins.sync_info.on_wait)
        ):
            hoist.append(ins)
        else:
            rest.append(ins)
    if not hoist:
        return
    tile_bb.instructions[:] = rest
    main_bb.instructions[:] = hoist + main_bb.instructions


def _build_program():
    nc = bass.Bass("TRN2", target_bir_lowering=False, debug=False)
    brep = nc.dram_tensor("brep", [128, N], FP16, kind="ExternalInput").ap()
    a2tf = nc.dram_tensor("a2tf", [128, NBLK * NPAIR], F32, kind="ExternalInput").ap()
    w2s = nc.dram_tensor("w2s", [128, 32 * NQ], FP16, kind="ExternalInput").ap()
    b2t = nc.dram_tensor("b2t", [128, 1], F32, kind="ExternalInput").ap()
    out = nc.dram_tensor("out", [RPC, N], FP16, kind="ExternalOutput").ap()

    with tile.TileContext(nc) as tc:
        with (
            tc.tile_pool(name="const", bufs=1) as cpool,
            tc.tile_pool(name="r", bufs=16) as rpool,
            tc.tile_pool(name="ps", bufs=1, space="PSUM") as pspool,
            tc.tile_pool(name="o", bufs=2) as opool,
        ):
            # Input loads split across the two HWDGE queues (sync + scalar) so
            # the transfers run in parallel; _hoist_input_dmas lifts these
            # above the start barrier.  brep (512 KB) is the critical tensor;
            # the scalar engine's program starts ~0.85us before sync's, so it
            # gets the bigger brep slice.  a2tf block-0 columns ride on scalar
            # right behind brep so the first pairs can start; w2s (matmuls) and
            # a2tf block-1 (needed ~32us in) trail on sync.
            BS = 1408  # brep split point: scalar gets [0:BS], sync [BS:N]
            brep_sb = cpool.tile([128, N], FP16)
            a2tf_sb = cpool.tile([128, RPC], F32)  # padded to keep SBUF layout
            w2s_sb = cpool.tile([128, 32 * NQ], FP16)
            b2_sb = cpool.tile([128, 1], F32)
            nc.scalar.dma_start(brep_sb[:, 0:BS], brep[:, 0:BS])
            nc.sync.dma_start(brep_sb[:, BS:N], brep[:, BS:N])
            nc.scalar.dma_start(a2tf_sb[:, 0:NPAIR], a2tf[:, 0:NPAIR])
            nc.sync.dma_start(w2s_sb[:], w2s[:])
            nc.scalar.dma_start(a2tf_sb[:, NPAIR : 2 * NPAIR], a2tf[:, NPAIR : 2 * NPAIR])
            nc.sync.dma_start(b2_sb[:], b2t[:])

            # Dependency-light dummy activation: walrus inserts the ACT table
            # load (~1.3us) right before the first ACTIVATE, after its waits.
            # Giving it a memset-initialized scrap input pulls the table load
            # into the boot window (~7us) instead of after the input DMAs.
            scrap = cpool.tile([128, 1], F32)
            nc.gpsimd.memset(scrap[:], 0.0)
            scrap_o = cpool.tile([128, 1], F32)
            nc.scalar.activation(
                scrap_o[:], scrap[:], mybir.ActivationFunctionType.Sigmoid
            )

            for b in range(NBLK):
                # Block 1 gets one PSUM tile per 512-col bank: each bank's
                # sigmoid then depends only on that bank's own stop-matmuls,
                # so the final sigmoid chain starts as soon as bank 0 finishes
                # instead of after the whole accumulation group.  Block 0's
                # sigmoids aren't latency-critical (they run during block 1),
                # so it uses a single [128, N] tile and ONE sigmoid op, which
                # costs 2.0us of ACT instead of 4 x 0.72us.
                if b == 0:
                    ps0 = pspool.tile([128, N], F32, name="ps0")
                    pout = lambda jt, g: ps0[
                        32 * g : 32 * g + 32, JT * jt : JT * (jt + 1)
                    ]
                else:
                    psb = [
                        pspool.tile([128, JT], F32, name=f"psb{b}_{jt}")
                        for jt in range(NJT)
                    ]
                    pout = lambda jt, g: psb[jt][32 * g : 32 * g + 32, :]
                for q in range(NQ):
                    rs = []
                    for g in range(4):
                        p = 4 * q + g
                        cp = b * NPAIR + p
                        r = rpool.tile([128, N], FP16)
                        if _use_act(b, p):
                            nc.scalar.activation(
                                r[:],
                                brep_sb[:],
                                mybir.ActivationFunctionType.Relu,
                                bias=a2tf_sb[:, cp : cp + 1],
                                scale=1.0,
                            )
                        else:
                            nc.vector.tensor_scalar(
                                out=r[:],
                                in0=brep_sb[:],
                                scalar1=a2tf_sb[:, cp : cp + 1],
                                scalar2=0.0,
                                op0=mybir.AluOpType.add,
                                op1=mybir.AluOpType.max,
                            )
                        rs.append(r)
                    for jt in range(NJT):
                        for g in range(4):
                            nc.tensor.matmul(
                                pout(jt, g),
                                w2s_sb[:, 32 * q : 32 * q + 32],
                                rs[g][:, JT * jt : JT * (jt + 1)],
                                start=(q == 0),
                                stop=(q == NQ - 1),
                                tile_position=(0, 32 * g),
                            )
                # fp16 output halves the store traffic; host casts back to f32
                # (sigmoid outputs live in [0, 1], fp16 rel err ~5e-4).
                o_sb = opool.tile([128, N], FP16)
                if b == 0:
                    # Mid-body: one big sigmoid + one store (cheapest on ACT).
                    nc.scalar.activation(
                        o_sb[:],
                        ps0[:],
                        mybir.ActivationFunctionType.Sigmoid,
                        bias=b2_sb[:, 0:1],
                        scale=1.0,
                    )
                    nc.sync.dma_start(out[0:128, :], o_sb[:])
                else:
                    # Tail: per-bank sigmoid + store chases the last matmuls.
                    for jt in range(NJT):
                        nc.scalar.activation(
                            o_sb[:, JT * jt : JT * (jt + 1)],
                            psb[jt][:],
                            mybir.ActivationFunctionType.Sigmoid,
                            bias=b2_sb[:, 0:1],
                            scale=1.0,
                        )
                        nc.sync.dma_start(
                            out[b * 128 : (b + 1) * 128, JT * jt : JT * (jt + 1)],
                            o_sb[:, JT * jt : JT * (jt + 1)],
                        )

    _split_sync_waits(nc)
    _hoist_input_dmas(nc)
    return nc


_NC_CACHE = None


def _get_program():
    global _NC_CACHE
    if _NC_CACHE is None:
        _NC_CACHE = _build_program()
    return _NC_CACHE


def _host_prep(Z, W1, b1, W2, b2):
    Z = np.asarray(Z, np.float64)
    W1 = np.asarray(W1, np.float64)
    b1 = np.asarray(b1, np.float64)
    W2 = np.asarray(W2, np.float64)
    b2 = np.asarray(b2, np.float64)

    A = Z @ W1[:D] + b1          # [N, H]
    Bm = Z @ W1[D:]              # [N, H]

    brep = np.empty((128, N), np.float16)
    brep[0:64] = Bm.T
    brep[64:128] = Bm.T

    # a2t: per core, column (b*64 + p) packs the biases of pair p of block b.
    a2tf = np.empty((NCORES, 128, NBLK * NPAIR), np.float32)
    for c in range(NCORES):
        for b in range(NBLK):
            for p in range(NPAIR):
                i0 = c * RPC + b * 128 + _PAIR_ROW0[p]
                cp = b * NPAIR + p
                a2tf[c, 0:64, cp] = A[i0]
                a2tf[c, 64:128, cp] = A[i0 + 1]

    # Zero-padded weight slots: slot s occupies columns [32s, 32s+32) and maps
    # contraction rows (2 x 64 hidden) to local output rows 2s, 2s+1.
    w2s = np.zeros((128, 32 * NQ), np.float16)
    w2c = W2[:, 0].astype(np.float16)
    for s in range(NQ):
        w2s[0:64, 32 * s + 2 * s] = w2c
        w2s[64:128, 32 * s + 2 * s + 1] = w2c

    b2t = np.full((128, 1), b2[0], np.float32)

    in_maps = []
    for c in range(NCORES):
        in_maps.append(
            {
                "brep": brep,
                "a2tf": np.ascontiguousarray(a2tf[c]),
                "w2s": w2s,
                "b2t": b2t,
            }
        )
    return in_maps


def _try_device_reset():
    """Recover wedged NeuronCores (NRT_EXEC_UNIT_UNRECOVERABLE) via the axon
    client's reset entry point.  Best-effort."""
    try:
        import ctypes

        import jax

        jax.devices()
        lib = ctypes.CDLL("/opt/axon/libaxon_pjrt.so")
        lib.axon_reset.restype = ctypes.c_int64
        lib.axon_reset()
        import time

        time.sleep(5)
    except Exception:
        pass


def run_kernel(Z, W1, b1, W2, b2, trace=False, **spmd_kwargs):
    """Run on the 8 NeuronCores; returns (pred [N, N] f32, BassKernelResults)."""
    nc = _get_program()
    in_maps = _host_prep(Z, W1, b1, W2, b2)
    try:
        res = run_bass_kernel_spmd(
            nc, in_maps, list(range(NCORES)), trace=trace, **spmd_kwargs
        )
    except Exception:
        _try_device_reset()
        res = run_bass_kernel_spmd(
            nc, in_maps, list(range(NCORES)), trace=trace, **spmd_kwargs
        )
    pred = np.concatenate(
        [res.results[c]["out"].astype(np.float32) for c in range(NCORES)], axis=0
    )
    return pred, res


def kernel(Z, W1, b1, W2, b2):
    pred, _ = run_kernel(Z, W1, b1, W2, b2)
    return pred


if __name__ == "__main__":
    rng = np.random.default_rng(0)
    Z = rng.standard_normal((N, D)).astype(np.float32)
    s1 = 1.0 / np.sqrt(2 * D)
    W1 = rng.uniform(-s1, s1, (2 * D, H)).astype(np.float32)
    b1 = rng.uniform(-s1, s1, (H,)).astype(np.float32)
    s2 = 1.0 / np.sqrt(H)
    W2 = rng.uniform(-s2, s2, (H, 1)).astype(np.float32)
    b2 = rng.uniform(-s2, s2, (1,)).astype(np.float32)
    pred = kernel(Z, W1, b1, W2, b2)
    print("pred", pred.shape, pred.dtype, pred[:2, :4])

